# revision 5
# baseline (speedup 1.0000x reference)
"""LocalizedFiltering (conv1->conv2->residual->RMSNorm) TRN2 Bass kernel.

Full inputs in, full outputs out. Internally: data-parallel over 8 NeuronCores,
2048 tokens per core (each of the 4 sequences of 4096 tokens is split in half;
even cores take sequence starts, odd cores the second halves).

Device layout is channel-major (tokens on the free dim), so matmul contraction
(channels) sits on partitions for both operands. Host transposes per-core input
slabs and output slabs. Matmuls run as float32r (TF32-class rounding, full PE
rate at N=512). The causal kernel-size-2 convs need one previous token (x) and
one previous conv1 output (o1) per shard: previous-x rows come in via the input
slab; previous-o1 is computed on device from the two extra x rows (N=1 matmuls
folded into the conv1 weight loop) and blended against the lf2 cache with a
per-core 0/1 scalar so sequence-start cores use the cache instead.
"""

import numpy as np
from contextlib import ExitStack

NCORES = 8
B, S, D = 4, 4096, 2048
DH = D // 2
T = (B * S) // NCORES  # tokens per core
EPS = 1e-6


# ---------------------------------------------------------------- device code


def build_module(D_, DH_, T_, NCH, EPS_CONST=EPS):
    """Build + compile the per-core Bass module. All dims in channel units;
    NCH = token chunk width for matmuls (free dim)."""
    import concourse.tile as tile
    from concourse import bacc, mybir

    f32 = mybir.dt.float32
    f32r = mybir.dt.float32r
    bf16 = mybir.dt.bfloat16

    nD = D_ // 128   # input-channel tiles (16)
    nE = DH_ // 128  # hidden-channel tiles (8)
    H = T_ // 2      # half size (1024)
    NCL = H // NCH   # chunks per half (2)

    nc = bacc.Bacc("TRN2", target_bir_lowering=False, debug=False)

    xT = nc.dram_tensor("xT", [D_, T_ + 2], f32r, kind="ExternalInput")
    w1t0 = nc.dram_tensor("w1t0", [nD, nE, 128, 128], f32r, kind="ExternalInput")
    w1t1 = nc.dram_tensor("w1t1", [nD, nE, 128, 128], f32r, kind="ExternalInput")
    w2t0 = nc.dram_tensor("w2t0", [nE, nD, 128, 128], f32r, kind="ExternalInput")
    w2t1 = nc.dram_tensor("w2t1", [nE, nD, 128, 128], f32r, kind="ExternalInput")
    b1v = nc.dram_tensor("b1v", [DH_], f32, kind="ExternalInput")
    b2v = nc.dram_tensor("b2v", [D_], f32, kind="ExternalInput")
    lnwv = nc.dram_tensor("lnwv", [D_], f32, kind="ExternalInput")
    # aux[:, e] (e<nE) = o1 cache col for hidden tile e (start cores: lf2; else 0)
    # aux[:, nE]       = blend scalar (start cores: 0.0, mid cores: 1.0)
    aux = nc.dram_tensor("aux", [128, nE + 1], f32, kind="ExternalInput")

    outT = nc.dram_tensor("outT", [D_, T_], f32, kind="ExternalOutput")
    o1last = nc.dram_tensor("o1last", [DH_], f32, kind="ExternalOutput")

    with tile.TileContext(nc) as tc:
        with ExitStack() as ctx:
            const = ctx.enter_context(tc.tile_pool(name="const", bufs=1))
            o1p = ctx.enter_context(tc.tile_pool(name="o1p", bufs=1))
            xhp = ctx.enter_context(tc.tile_pool(name="xhp", bufs=1))
            wp = ctx.enter_context(tc.tile_pool(name="wp", bufs=6))
            yp = ctx.enter_context(tc.tile_pool(name="yp", bufs=1))
            smallp = ctx.enter_context(tc.tile_pool(name="smallp", bufs=2))
            invp = ctx.enter_context(tc.tile_pool(name="invp", bufs=2))
            ysqp = ctx.enter_context(tc.tile_pool(name="ysqp", bufs=3))
            outp = ctx.enter_context(tc.tile_pool(name="outp", bufs=3))
            ps_o1 = ctx.enter_context(tc.tile_pool(name="ps_o1", bufs=2, space="PSUM"))
            ps_y = ctx.enter_context(tc.tile_pool(name="ps_y", bufs=3, space="PSUM"))
            ps_ssq = ctx.enter_context(tc.tile_pool(name="ps_ssq", bufs=2, space="PSUM"))
            ps_tail = ctx.enter_context(tc.tile_pool(name="ps_tail", bufs=1, space="PSUM"))

            # constants
            b1sb = const.tile([128, nE], f32, tag="b1sb")
            b2sb = const.tile([128, nD], f32, tag="b2sb")
            lnwsb = const.tile([128, nD], f32, tag="lnwsb")
            auxsb = const.tile([128, nE + 1], f32, tag="auxsb")
            ones = const.tile([128, 128], bf16, tag="ones")
            epssb = const.tile([128, 1], f32, tag="epssb")
            nc.vector.memset(epssb[:], EPS_CONST)
            nc.sync.dma_start(out=b1sb[:], in_=b1v.ap().rearrange("(e p) -> p e", p=128))
            nc.sync.dma_start(out=b2sb[:], in_=b2v.ap().rearrange("(e p) -> p e", p=128))
            nc.sync.dma_start(out=lnwsb[:], in_=lnwv.ap().rearrange("(e p) -> p e", p=128))
            nc.sync.dma_start(out=auxsb[:], in_=aux.ap())
            nc.vector.memset(ones[:], 1.0)

            # persistent conv1 output, channel-major [DH_, T_+1]
            o1T = [o1p.tile([128, T_ + 1], f32r, tag=f"o1_{e}", name=f"o1_{e}")
                   for e in range(nE)]

            ptail = ps_tail.tile([128, 16], f32, tag="ptail", name="ptail")

            for h in range(2):
                # ---- load x half: cols [h*H, h*H + H + 2) of xT
                xh = []
                for d in range(nD):
                    t = xhp.tile([128, H + 2], f32r, tag=f"xh_{d}", name=f"xh_{d}")
                    nc.sync.dma_start(
                        out=t[:], in_=xT.ap()[d * 128:(d + 1) * 128, h * H: h * H + H + 2]
                    )
                    xh.append(t)

                # ---- conv1: o1T cols [h*H + 1 + cl*NCH, +NCH) per chunk
                for e in range(nE):
                    pss = [ps_o1.tile([128, NCH], f32, tag="ps_o1", name="ps_o1") for _ in range(NCL)]
                    first = True
                    for d in range(nD):
                        for tap, wsrc in ((0, w1t0), (1, w1t1)):
                            wt = wp.tile([128, 128], f32r, tag="w")
                            nc.sync.dma_start(out=wt[:], in_=wsrc.ap()[d, e])
                            if h == 0:
                                # o1_prev tail: token t0-1 needs x[t0-2], x[t0-1].
                                # N=2 (second col is a discarded duplicate):
                                # N=1 fp32r matmuls fail the walrus ISA check.
                                nc.tensor.matmul(
                                    ptail[:, 2 * e:2 * e + 2], wt[:],
                                    xh[d][:, tap:tap + 2],
                                    start=(e == 0 and first), stop=False,
                                )
                            for cl in range(NCL):
                                k0 = cl * NCH + 1 + tap
                                nc.tensor.matmul(
                                    pss[cl][:], wt[:], xh[d][:, k0:k0 + NCH],
                                    start=first, stop=(d == nD - 1 and tap == 1),
                                )
                            first = False
                    for cl in range(NCL):
                        j0 = h * H + 1 + cl * NCH
                        nc.scalar.activation(
                            out=o1T[e][:, j0:j0 + NCH], in_=pss[cl][:],
                            func=mybir.ActivationFunctionType.Identity,
                            bias=b1sb[:, e:e + 1], scale=1.0,
                        )

                if h == 0:
                    # blend o1_prev with cache: o1T[:,0] = sc*(tail+b1) + o1c
                    for e in range(nE):
                        tb = smallp.tile([128, 1], f32, tag="tb")
                        nc.scalar.activation(
                            out=tb[:], in_=ptail[:, 2 * e:2 * e + 1],
                            func=mybir.ActivationFunctionType.Identity,
                            bias=b1sb[:, e:e + 1], scale=1.0,
                        )
                        nc.vector.scalar_tensor_tensor(
                            out=o1T[e][:, 0:1], in0=tb[:],
                            scalar=auxsb[:, nE:nE + 1], in1=auxsb[:, e:e + 1],
                            op0=mybir.AluOpType.mult, op1=mybir.AluOpType.add,
                        )

                # ---- conv2 + residual + RMSNorm per chunk
                for cl in range(NCL):
                    J0 = h * H + cl * NCH
                    pssq = ps_ssq.tile([128, NCH], f32, tag="ps_ssq")
                    ys = []
                    for dout in range(nD):
                        py = ps_y.tile([128, NCH], f32, tag="ps_y")
                        first = True
                        for e in range(nE):
                            for tap, wsrc in ((0, w2t0), (1, w2t1)):
                                wt = wp.tile([128, 128], f32r, tag="w")
                                nc.sync.dma_start(out=wt[:], in_=wsrc.ap()[e, dout])
                                nc.tensor.matmul(
                                    py[:], wt[:], o1T[e][:, J0 + tap:J0 + tap + NCH],
                                    start=first, stop=(e == nE - 1 and tap == 1),
                                )
                                first = False
                        # y = (psum + b2) + x
                        yt = yp.tile([128, NCH], f32, tag=f"y_{dout}")
                        k0 = cl * NCH + 2
                        nc.vector.scalar_tensor_tensor(
                            out=yt[:], in0=py[:], scalar=b2sb[:, dout:dout + 1],
                            in1=xh[dout][:, k0:k0 + NCH].bitcast(f32),
                            op0=mybir.AluOpType.add, op1=mybir.AluOpType.add,
                        )
                        ys.append(yt)
                        ysq = ysqp.tile([128, NCH], bf16, tag="ysq")
                        nc.vector.tensor_mul(ysq[:], yt[:], yt[:])
                        nc.tensor.matmul(
                            pssq[:], ones[:], ysq[:],
                            start=(dout == 0), stop=(dout == nD - 1),
                        )
                    # inv_rms = 1/sqrt(mean + eps), replicated on all partitions
                    st = invp.tile([128, NCH], f32, tag="st")
                    nc.scalar.activation(
                        out=st[:], in_=pssq[:],
                        func=mybir.ActivationFunctionType.Sqrt,
                        bias=epssb[:], scale=1.0 / D_,
                    )
                    inv = invp.tile([128, NCH], f32, tag="inv")
                    nc.vector.reciprocal(inv[:], st[:])
                    for dout in range(nD):
                        ot = outp.tile([128, NCH], f32, tag="ot")
                        nc.vector.scalar_tensor_tensor(
                            out=ot[:], in0=ys[dout][:], scalar=lnwsb[:, dout:dout + 1],
                            in1=inv[:],
                            op0=mybir.AluOpType.mult, op1=mybir.AluOpType.mult,
                        )
                        nc.sync.dma_start(
                            out=outT.ap()[dout * 128:(dout + 1) * 128, J0:J0 + NCH],
                            in_=ot[:],
                        )

            # last conv1 state (token t0+T-1) for the lf2 cache output
            for e in range(nE):
                nc.sync.dma_start(
                    out=o1last.ap().rearrange("(e p) -> p e", p=128)[:, e:e + 1],
                    in_=o1T[e][:, T_:T_ + 1].bitcast(f32),
                )

    nc.compile()
    return nc


# ------------------------------------------------------------------ host glue


def _pack_w(wT, nk, nm):
    # [K, M] -> [nk, nm, 128, 128] tile-major, contiguous
    K, M = wT.shape
    return np.ascontiguousarray(
        wT.reshape(nk, 128, nm, 128).transpose(0, 2, 1, 3)
    )


def prepare_core_inputs(x3, lf1_cache, lf2_cache, w1, b1, w2, b2, ln_w,
                        ncores, S_, D_, DH_):
    """Build per-core in_maps. x3: [B, S, D] float32."""
    nD = D_ // 128
    nE = DH_ // 128
    B_ = x3.shape[0]
    T_ = (B_ * S_) // ncores
    per_seq = S_ // T_  # cores per sequence

    w1p0 = _pack_w(np.ascontiguousarray(w1[:, :, 0].T), nD, nE)
    w1p1 = _pack_w(np.ascontiguousarray(w1[:, :, 1].T), nD, nE)
    w2p0 = _pack_w(np.ascontiguousarray(w2[:, :, 0].T), nE, nD)
    w2p1 = _pack_w(np.ascontiguousarray(w2[:, :, 1].T), nE, nD)
    b1c = np.ascontiguousarray(b1, np.float32)
    b2c = np.ascontiguousarray(b2, np.float32)
    lnc = np.ascontiguousarray(ln_w, np.float32)

    in_maps = []
    for c in range(ncores):
        b = c // per_seq
        part = c % per_seq
        t0 = part * T_
        x_ext = np.empty((T_ + 2, D_), np.float32)
        aux = np.zeros((128, nE + 1), np.float32)
        if part == 0:
            x_ext[0] = 0.0
            x_ext[1] = lf1_cache[b, :, 0, 0]
            aux[:, :nE] = lf2_cache[b, :, 0, 0].reshape(nE, 128).T
            aux[:, nE] = 0.0
        else:
            x_ext[0] = x3[b, t0 - 2]
            x_ext[1] = x3[b, t0 - 1]
            aux[:, nE] = 1.0
        x_ext[2:] = x3[b, t0:t0 + T_]
        xT = np.ascontiguousarray(x_ext.T)
        in_maps.append({
            "xT": xT, "w1t0": w1p0, "w1t1": w1p1, "w2t0": w2p0, "w2t1": w2p1,
            "b1v": b1c, "b2v": b2c, "lnwv": lnc, "aux": aux,
        })
    return in_maps


_CACHE = {}


def _get_module():
    key = (D, DH, T)
    if key not in _CACHE:
        _CACHE[key] = build_module(D, DH, T, 512)
    return _CACHE[key]


def kernel(inputs, lf1_cache, lf2_cache, w1, b1, w2, b2, ln_w):
    from concourse.bass_utils import run_bass_kernel_spmd

    x = np.asarray(inputs, np.float32)
    lf1_cache = np.asarray(lf1_cache, np.float32)
    lf2_cache = np.asarray(lf2_cache, np.float32)
    w1 = np.asarray(w1, np.float32)
    b1 = np.asarray(b1, np.float32)
    w2 = np.asarray(w2, np.float32)
    b2 = np.asarray(b2, np.float32)
    ln_w = np.asarray(ln_w, np.float32)

    x3 = x.reshape(B, S, D)
    in_maps = prepare_core_inputs(x3, lf1_cache, lf2_cache, w1, b1, w2, b2,
                                  ln_w, NCORES, S, D, DH)
    nc = _get_module()
    res = run_bass_kernel_spmd(nc, in_maps, core_ids=list(range(NCORES)))

    per_seq = S // T
    lf_output = np.empty((B, S, D), np.float32)
    lf2 = np.empty((B, DH, 1, 1), np.float32)
    for c in range(NCORES):
        b = c // per_seq
        part = c % per_seq
        t0 = part * T
        lf_output[b, t0:t0 + T] = res.results[c]["outT"].T
        if part == per_seq - 1:
            lf2[b, :, 0, 0] = res.results[c]["o1last"]
    lf1 = np.ascontiguousarray(x3[:, -1][:, :, None, None])
    return lf_output, lf1, lf2


# revision 6
# speedup vs baseline: 1.0181x; 1.0181x over previous
"""LocalizedFiltering (conv1->conv2->residual->RMSNorm) TRN2 Bass kernel.

Full inputs in, full outputs out. Internally: data-parallel over 8 NeuronCores,
2048 tokens per core (each of the 4 sequences of 4096 tokens is split in half;
even cores take sequence starts, odd cores the second halves).

Device layout is channel-major (tokens on the free dim), so matmul contraction
(channels) sits on partitions for both operands. Host transposes per-core input
slabs and output slabs. Matmuls run as float32r (TF32-class rounding, full PE
rate at N=512). The causal kernel-size-2 convs need one previous token (x) and
one previous conv1 output (o1) per shard: previous-x rows come in via the input
slab; previous-o1 is computed on device from the two extra x rows (N=2 matmuls
folded into the conv1 weight loop) and blended against the lf2 cache with a
per-core 0/1 scalar so sequence-start cores use the cache instead. RMSNorm's
cross-partition sum uses a ones-matmul (bf16) accumulating into PSUM, which
also replicates the per-token sum across all partitions for the final scale.
"""

import numpy as np
from contextlib import ExitStack

NCORES = 8
B, S, D = 4, 4096, 2048
DH = D // 2
T = (B * S) // NCORES  # tokens per core
EPS = 1e-6


# ---------------------------------------------------------------- device code


def build_module(D_, DH_, T_, NCH, EPS_CONST=EPS):
    """Build + compile the per-core Bass module. All dims in channel units;
    NCH = token chunk width for matmuls (free dim)."""
    import concourse.tile as tile
    from concourse import bacc, mybir

    f32 = mybir.dt.float32
    f32r = mybir.dt.float32r
    bf16 = mybir.dt.bfloat16
    ADD = mybir.AluOpType.add
    MUL = mybir.AluOpType.mult

    nD = D_ // 128   # input-channel tiles (16)
    nE = DH_ // 128  # hidden-channel tiles (8)
    H = T_ // 2      # half size (1024)
    NCL = H // NCH   # chunks per half (2)

    nc = bacc.Bacc("TRN2", target_bir_lowering=False, debug=False)

    xT = nc.dram_tensor("xT", [D_, T_ + 2], f32r, kind="ExternalInput")
    # weights pre-packed on host, lhsT tile-major:
    # w1pk[e, tap, d, p, m] = w1[e*128+m, d*128+p, tap]
    w1pk = nc.dram_tensor("w1pk", [nE, 2, nD, 128, 128], f32r, kind="ExternalInput")
    # w2pk[do, tap, e, p, m] = w2[do*128+m, e*128+p, tap]
    w2pk = nc.dram_tensor("w2pk", [nD, 2, nE, 128, 128], f32r, kind="ExternalInput")
    b1v = nc.dram_tensor("b1v", [DH_], f32, kind="ExternalInput")
    b2v = nc.dram_tensor("b2v", [D_], f32, kind="ExternalInput")
    lnwv = nc.dram_tensor("lnwv", [D_], f32, kind="ExternalInput")
    # aux[:, e] (e<nE) = o1 cache col for hidden tile e (start cores: lf2; else 0)
    # aux[:, nE]       = blend scalar (start cores: 0.0, mid cores: 1.0)
    aux = nc.dram_tensor("aux", [128, nE + 1], f32, kind="ExternalInput")

    outT = nc.dram_tensor("outT", [D_, T_], f32, kind="ExternalOutput")
    o1last = nc.dram_tensor("o1last", [DH_], f32, kind="ExternalOutput")

    with tile.TileContext(nc) as tc:
        with ExitStack() as ctx:
            const = ctx.enter_context(tc.tile_pool(name="const", bufs=1))
            o1p = ctx.enter_context(tc.tile_pool(name="o1p", bufs=1))
            xhp = ctx.enter_context(tc.tile_pool(name="xhp", bufs=1))
            w1p = ctx.enter_context(tc.tile_pool(name="w1p", bufs=2))
            w2p = ctx.enter_context(tc.tile_pool(name="w2p", bufs=3))
            yp = ctx.enter_context(tc.tile_pool(name="yp", bufs=1))
            smallp = ctx.enter_context(tc.tile_pool(name="smallp", bufs=2))
            invp = ctx.enter_context(tc.tile_pool(name="invp", bufs=2))
            ysqp = ctx.enter_context(tc.tile_pool(name="ysqp", bufs=3))
            outp = ctx.enter_context(tc.tile_pool(name="outp", bufs=3))
            ps_o1 = ctx.enter_context(tc.tile_pool(name="ps_o1", bufs=2, space="PSUM"))
            ps_y = ctx.enter_context(tc.tile_pool(name="ps_y", bufs=3, space="PSUM"))
            ps_ssq = ctx.enter_context(tc.tile_pool(name="ps_ssq", bufs=2, space="PSUM"))
            ps_tail = ctx.enter_context(tc.tile_pool(name="ps_tail", bufs=1, space="PSUM"))

            # constants
            b1sb = const.tile([128, nE], f32, tag="b1sb")
            b2sb = const.tile([128, nD], f32, tag="b2sb")
            lnwsb = const.tile([128, nD], f32, tag="lnwsb")
            auxsb = const.tile([128, nE + 1], f32, tag="auxsb")
            ones = const.tile([128, 128], bf16, tag="ones")
            epssb = const.tile([128, 1], f32, tag="epssb")
            nc.vector.memset(epssb[:], EPS_CONST)
            nc.sync.dma_start(out=b1sb[:], in_=b1v.ap().rearrange("(e p) -> p e", p=128))
            nc.sync.dma_start(out=b2sb[:], in_=b2v.ap().rearrange("(e p) -> p e", p=128))
            nc.sync.dma_start(out=lnwsb[:], in_=lnwv.ap().rearrange("(e p) -> p e", p=128))
            nc.sync.dma_start(out=auxsb[:], in_=aux.ap())
            nc.vector.memset(ones[:], 1.0)

            # conv1 output for the current half, channel-major [DH_, H+1];
            # col 0 = previous token's o1 (blend for h0, chained from col H after)
            o1T = [o1p.tile([128, H + 1], f32r, tag=f"o1_{e}", name=f"o1_{e}")
                   for e in range(nE)]

            ptail = ps_tail.tile([128, 16], f32, tag="ptail", name="ptail")

            for h in range(2):
                # ---- load x half: cols [h*H, h*H + H + 2) of xT
                xh = []
                for d in range(nD):
                    t = xhp.tile([128, H + 2], f32r, tag=f"xh_{d}", name=f"xh_{d}")
                    nc.sync.dma_start(
                        out=t[:], in_=xT.ap()[d * 128:(d + 1) * 128, h * H: h * H + H + 2]
                    )
                    xh.append(t)

                # ---- conv1: o1T local cols [1 + cl*NCH, +NCH) per chunk
                for e in range(nE):
                    wb = []
                    for tap in (0, 1):
                        w = w1p.tile([128, nD, 128], f32r, tag="w1b", name="w1b")
                        nc.gpsimd.dma_start(
                            out=w[:], in_=w1pk.ap()[e, tap].rearrange("d p m -> p d m")
                        )
                        wb.append(w)
                    pss = [ps_o1.tile([128, NCH], f32, tag="ps_o1", name="ps_o1")
                           for _ in range(NCL)]
                    first = True
                    for d in range(nD):
                        for tap in (0, 1):
                            wt = wb[tap][:, d, :]
                            if h == 0:
                                # o1_prev tail: token t0-1 needs x[t0-2], x[t0-1].
                                # N=2 (2nd col discarded): N=1 fp32r fails ISA check.
                                nc.tensor.matmul(
                                    ptail[:, 2 * e:2 * e + 2], wt,
                                    xh[d][:, tap:tap + 2],
                                    start=(e == 0 and first), stop=False,
                                )
                            for cl in range(NCL):
                                k0 = cl * NCH + 1 + tap
                                nc.tensor.matmul(
                                    pss[cl][:], wt, xh[d][:, k0:k0 + NCH],
                                    start=first, stop=(d == nD - 1 and tap == 1),
                                )
                            first = False
                    for cl in range(NCL):
                        nc.vector.tensor_scalar_add(
                            o1T[e][:, 1 + cl * NCH:1 + (cl + 1) * NCH], pss[cl][:],
                            b1sb[:, e:e + 1],
                        )

                if h == 0:
                    # blend o1_prev with cache: o1T[:,0] = sc*(tail+b1) + o1c
                    for e in range(nE):
                        tb = smallp.tile([128, 1], f32, tag="tb")
                        nc.vector.tensor_scalar_add(
                            tb[:], ptail[:, 2 * e:2 * e + 1], b1sb[:, e:e + 1]
                        )
                        nc.vector.scalar_tensor_tensor(
                            out=o1T[e][:, 0:1], in0=tb[:],
                            scalar=auxsb[:, nE:nE + 1], in1=auxsb[:, e:e + 1],
                            op0=MUL, op1=ADD,
                        )

                # ---- conv2 + residual + RMSNorm per chunk
                for cl in range(NCL):
                    J0g = h * H + cl * NCH   # global output col
                    J0 = cl * NCH            # local o1T col
                    pssq = ps_ssq.tile([128, NCH], f32, tag="ps_ssq", name="ps_ssq")
                    ys = []
                    for dout in range(nD):
                        wb2 = []
                        for tap in (0, 1):
                            w = w2p.tile([128, nE, 128], f32r, tag="w2b", name="w2b")
                            nc.gpsimd.dma_start(
                                out=w[:],
                                in_=w2pk.ap()[dout, tap].rearrange("e p m -> p e m"),
                            )
                            wb2.append(w)
                        py = ps_y.tile([128, NCH], f32, tag="ps_y", name="ps_y")
                        first = True
                        for e in range(nE):
                            for tap in (0, 1):
                                nc.tensor.matmul(
                                    py[:], wb2[tap][:, e, :],
                                    o1T[e][:, J0 + tap:J0 + tap + NCH],
                                    start=first, stop=(e == nE - 1 and tap == 1),
                                )
                                first = False
                        # y = (psum + b2) + x
                        yt = yp.tile([128, NCH], f32, tag=f"y_{dout}", name=f"y_{dout}")
                        k0 = cl * NCH + 2
                        nc.vector.scalar_tensor_tensor(
                            out=yt[:], in0=py[:], scalar=b2sb[:, dout:dout + 1],
                            in1=xh[dout][:, k0:k0 + NCH].bitcast(f32),
                            op0=ADD, op1=ADD,
                        )
                        ys.append(yt)
                        ysq = ysqp.tile([128, NCH], bf16, tag="ysq", name="ysq")
                        nc.vector.tensor_mul(ysq[:], yt[:], yt[:])
                        nc.tensor.matmul(
                            pssq[:], ones[:], ysq[:],
                            start=(dout == 0), stop=(dout == nD - 1),
                        )
                    # inv_rms = 1/sqrt(mean + eps), replicated on all partitions
                    st = invp.tile([128, NCH], f32, tag="st", name="st")
                    nc.scalar.activation(
                        out=st[:], in_=pssq[:],
                        func=mybir.ActivationFunctionType.Sqrt,
                        bias=epssb[:], scale=1.0 / D_,
                    )
                    inv = invp.tile([128, NCH], f32, tag="inv", name="inv")
                    nc.vector.reciprocal(inv[:], st[:])
                    for dout in range(nD):
                        ot = outp.tile([128, NCH], f32, tag="ot", name="ot")
                        nc.vector.scalar_tensor_tensor(
                            out=ot[:], in0=ys[dout][:], scalar=lnwsb[:, dout:dout + 1],
                            in1=inv[:], op0=MUL, op1=MUL,
                        )
                        nc.sync.dma_start(
                            out=outT.ap()[dout * 128:(dout + 1) * 128, J0g:J0g + NCH],
                            in_=ot[:],
                        )

                if h == 0:
                    # chain the half boundary: o1(t0+H-1) -> col 0 for half 1
                    for e in range(nE):
                        nc.vector.tensor_copy(o1T[e][:, 0:1], o1T[e][:, H:H + 1])

            # last conv1 state (token t0+T-1) for the lf2 cache output
            for e in range(nE):
                nc.sync.dma_start(
                    out=o1last.ap().rearrange("(e p) -> p e", p=128)[:, e:e + 1],
                    in_=o1T[e][:, H:H + 1].bitcast(f32),
                )

    nc.compile()
    return nc


# ------------------------------------------------------------------ host glue


def prepare_core_inputs(x3, lf1_cache, lf2_cache, w1, b1, w2, b2, ln_w,
                        ncores, S_, D_, DH_):
    """Build per-core in_maps. x3: [B, S, D] float32."""
    nD = D_ // 128
    nE = DH_ // 128
    B_ = x3.shape[0]
    T_ = (B_ * S_) // ncores
    per_seq = S_ // T_  # cores per sequence

    # lhsT tile-major packs (see build_module comments)
    w1pk = np.ascontiguousarray(
        w1.reshape(nE, 128, nD, 128, 2).transpose(0, 4, 2, 3, 1).astype(np.float32)
    )
    w2pk = np.ascontiguousarray(
        w2.reshape(nD, 128, nE, 128, 2).transpose(0, 4, 2, 3, 1).astype(np.float32)
    )
    b1c = np.ascontiguousarray(b1, np.float32)
    b2c = np.ascontiguousarray(b2, np.float32)
    lnc = np.ascontiguousarray(ln_w, np.float32)

    in_maps = []
    for c in range(ncores):
        b = c // per_seq
        part = c % per_seq
        t0 = part * T_
        x_ext = np.empty((T_ + 2, D_), np.float32)
        aux = np.zeros((128, nE + 1), np.float32)
        if part == 0:
            x_ext[0] = 0.0
            x_ext[1] = lf1_cache[b, :, 0, 0]
            aux[:, :nE] = lf2_cache[b, :, 0, 0].reshape(nE, 128).T
            aux[:, nE] = 0.0
        else:
            x_ext[0] = x3[b, t0 - 2]
            x_ext[1] = x3[b, t0 - 1]
            aux[:, nE] = 1.0
        x_ext[2:] = x3[b, t0:t0 + T_]
        xT = np.ascontiguousarray(x_ext.T)
        in_maps.append({
            "xT": xT, "w1pk": w1pk, "w2pk": w2pk,
            "b1v": b1c, "b2v": b2c, "lnwv": lnc, "aux": aux,
        })
    return in_maps


_CACHE = {}


def _get_module():
    key = (D, DH, T)
    if key not in _CACHE:
        _CACHE[key] = build_module(D, DH, T, 512)
    return _CACHE[key]


def kernel(inputs, lf1_cache, lf2_cache, w1, b1, w2, b2, ln_w):
    from concourse.bass_utils import run_bass_kernel_spmd

    x = np.asarray(inputs, np.float32)
    lf1_cache = np.asarray(lf1_cache, np.float32)
    lf2_cache = np.asarray(lf2_cache, np.float32)
    w1 = np.asarray(w1, np.float32)
    b1 = np.asarray(b1, np.float32)
    w2 = np.asarray(w2, np.float32)
    b2 = np.asarray(b2, np.float32)
    ln_w = np.asarray(ln_w, np.float32)

    x3 = x.reshape(B, S, D)
    in_maps = prepare_core_inputs(x3, lf1_cache, lf2_cache, w1, b1, w2, b2,
                                  ln_w, NCORES, S, D, DH)
    nc = _get_module()
    res = run_bass_kernel_spmd(nc, in_maps, core_ids=list(range(NCORES)))

    per_seq = S // T
    lf_output = np.empty((B, S, D), np.float32)
    lf2 = np.empty((B, DH, 1, 1), np.float32)
    for c in range(NCORES):
        b = c // per_seq
        part = c % per_seq
        t0 = part * T
        lf_output[b, t0:t0 + T] = res.results[c]["outT"].T
        if part == per_seq - 1:
            lf2[b, :, 0, 0] = res.results[c]["o1last"]
    lf1 = np.ascontiguousarray(x3[:, -1][:, :, None, None])
    return lf_output, lf1, lf2


# revision 8
# speedup vs baseline: 1.7546x; 1.7234x over previous
"""LocalizedFiltering (conv1->conv2->residual->RMSNorm) TRN2 Bass kernel.

Full inputs in, full outputs out. Internally: data-parallel over 8 NeuronCores,
2048 tokens per core (each of the 4 sequences of 4096 tokens is split in half;
even cores take sequence starts, odd cores the second halves).

Device layout is channel-major (tokens on the free dim), so matmul contraction
(channels) sits on partitions for both operands. Host transposes per-core input
slabs and output slabs. Matmuls run as float32r (TF32-class rounding, full PE
rate at N=512). The causal kernel-size-2 convs need one previous token (x) and
one previous conv1 output (o1) per shard: previous-x rows come in via the input
slab; previous-o1 is computed on device from the two extra x rows (N=2 matmuls
folded into the conv1 weight loop) and blended against the lf2 cache with a
per-core 0/1 scalar so sequence-start cores use the cache instead. RMSNorm's
cross-partition sum uses a ones-matmul (bf16) accumulating into PSUM, which
also replicates the per-token sum across all partitions for the final scale.
"""

import numpy as np
from contextlib import ExitStack

NCORES = 8
B, S, D = 4, 4096, 2048
DH = D // 2
T = (B * S) // NCORES  # tokens per core
EPS = 1e-6


# ---------------------------------------------------------------- device code


def build_module(D_, DH_, T_, NCH, EPS_CONST=EPS):
    """Build + compile the per-core Bass module. All dims in channel units;
    NCH = token chunk width for matmuls (free dim)."""
    import concourse.tile as tile
    from concourse import bacc, mybir

    f32 = mybir.dt.float32
    f32r = mybir.dt.float32r
    bf16 = mybir.dt.bfloat16
    ADD = mybir.AluOpType.add
    MUL = mybir.AluOpType.mult

    nD = D_ // 128   # input-channel tiles (16)
    nE = DH_ // 128  # hidden-channel tiles (8)
    H = T_ // 2      # half size (1024)
    NCL = H // NCH   # chunks per half (2)

    nc = bacc.Bacc("TRN2", target_bir_lowering=False, debug=False)

    xT = nc.dram_tensor("xT", [D_, T_ + 2], f32r, kind="ExternalInput")
    # weights pre-packed on host, lhsT tile-major:
    # w1pk[e, tap, d, p, m] = w1[e*128+m, d*128+p, tap]
    w1pk = nc.dram_tensor("w1pk", [nE, 2, nD, 128, 128], f32r, kind="ExternalInput")
    # w2pk[do, tap, e, p, m] = w2[do*128+m, e*128+p, tap]
    w2pk = nc.dram_tensor("w2pk", [nD, 2, nE, 128, 128], f32r, kind="ExternalInput")
    b1v = nc.dram_tensor("b1v", [DH_], f32, kind="ExternalInput")
    b2v = nc.dram_tensor("b2v", [D_], f32, kind="ExternalInput")
    lnwv = nc.dram_tensor("lnwv", [D_], f32, kind="ExternalInput")
    # aux[:, e] (e<nE) = o1 cache col for hidden tile e (start cores: lf2; else 0)
    # aux[:, nE]       = blend scalar (start cores: 0.0, mid cores: 1.0)
    aux = nc.dram_tensor("aux", [128, nE + 1], f32, kind="ExternalInput")

    outT = nc.dram_tensor("outT", [D_, T_], f32, kind="ExternalOutput")
    o1last = nc.dram_tensor("o1last", [DH_], f32, kind="ExternalOutput")

    with tile.TileContext(nc) as tc:
        with ExitStack() as ctx:
            const = ctx.enter_context(tc.tile_pool(name="const", bufs=1))
            o1p = ctx.enter_context(tc.tile_pool(name="o1p", bufs=1))
            xhp = ctx.enter_context(tc.tile_pool(name="xhp", bufs=1))
            w1p = ctx.enter_context(tc.tile_pool(name="w1p", bufs=2))
            w2p = ctx.enter_context(tc.tile_pool(name="w2p", bufs=4))
            yp = ctx.enter_context(tc.tile_pool(name="yp", bufs=1))
            smallp = ctx.enter_context(tc.tile_pool(name="smallp", bufs=2))
            invp = ctx.enter_context(tc.tile_pool(name="invp", bufs=2))
            ysqp = ctx.enter_context(tc.tile_pool(name="ysqp", bufs=2))
            outp = ctx.enter_context(tc.tile_pool(name="outp", bufs=2))
            ps_o1 = ctx.enter_context(tc.tile_pool(name="ps_o1", bufs=2, space="PSUM"))
            ps_y = ctx.enter_context(tc.tile_pool(name="ps_y", bufs=3, space="PSUM"))
            ps_ssq = ctx.enter_context(tc.tile_pool(name="ps_ssq", bufs=2, space="PSUM"))
            ps_tail = ctx.enter_context(tc.tile_pool(name="ps_tail", bufs=1, space="PSUM"))

            # constants
            b1sb = const.tile([128, nE], f32, tag="b1sb")
            b2sb = const.tile([128, nD], f32, tag="b2sb")
            lnwsb = const.tile([128, nD], f32, tag="lnwsb")
            auxsb = const.tile([128, nE + 1], f32, tag="auxsb")
            ones = const.tile([128, 128], bf16, tag="ones")
            epssb = const.tile([128, 1], f32, tag="epssb")
            nc.vector.memset(epssb[:], EPS_CONST)
            nc.sync.dma_start(out=b1sb[:], in_=b1v.ap().rearrange("(e p) -> p e", p=128))
            nc.sync.dma_start(out=b2sb[:], in_=b2v.ap().rearrange("(e p) -> p e", p=128))
            nc.sync.dma_start(out=lnwsb[:], in_=lnwv.ap().rearrange("(e p) -> p e", p=128))
            nc.sync.dma_start(out=auxsb[:], in_=aux.ap())
            nc.vector.memset(ones[:], 1.0)

            # conv1 output for the current half, channel-major [DH_, H+1];
            # col 0 = previous token's o1 (blend for h0, chained from col H after)
            o1T = [o1p.tile([128, H + 1], f32r, tag=f"o1_{e}", name=f"o1_{e}")
                   for e in range(nE)]

            ptail = ps_tail.tile([128, 16], f32, tag="ptail", name="ptail")

            for h in range(2):
                # ---- load x half: cols [h*H, h*H + H + 2) of xT
                xh = []
                for d in range(nD):
                    t = xhp.tile([128, H + 2], f32r, tag=f"xh_{d}", name=f"xh_{d}")
                    nc.sync.dma_start(
                        out=t[:], in_=xT.ap()[d * 128:(d + 1) * 128, h * H: h * H + H + 2]
                    )
                    xh.append(t)

                # ---- conv1: o1T local cols [1 + cl*NCH, +NCH) per chunk
                for e in range(nE):
                    wb = w1p.tile([128, 2 * nD, 128], f32r, tag="w1b", name="w1b")
                    nc.gpsimd.dma_start(
                        out=wb[:], in_=w1pk.ap()[e].rearrange("t d p m -> p (t d) m")
                    )
                    pss = [ps_o1.tile([128, NCH], f32, tag="ps_o1", name="ps_o1")
                           for _ in range(NCL)]
                    first = True
                    for d in range(nD):
                        for tap in (0, 1):
                            wt = wb[:, tap * nD + d, :]
                            if h == 0:
                                # o1_prev tail: token t0-1 needs x[t0-2], x[t0-1].
                                # N=2 (2nd col discarded): N=1 fp32r fails ISA check.
                                nc.tensor.matmul(
                                    ptail[:, 2 * e:2 * e + 2], wt,
                                    xh[d][:, tap:tap + 2],
                                    start=(e == 0 and first), stop=False,
                                )
                            for cl in range(NCL):
                                k0 = cl * NCH + 1 + tap
                                nc.tensor.matmul(
                                    pss[cl][:], wt, xh[d][:, k0:k0 + NCH],
                                    start=first, stop=(d == nD - 1 and tap == 1),
                                )
                            first = False
                    for cl in range(NCL):
                        nc.vector.tensor_scalar_add(
                            o1T[e][:, 1 + cl * NCH:1 + (cl + 1) * NCH], pss[cl][:],
                            b1sb[:, e:e + 1],
                        )

                if h == 0:
                    # blend o1_prev with cache: o1T[:,0] = sc*(tail+b1) + o1c
                    for e in range(nE):
                        tb = smallp.tile([128, 1], f32, tag="tb")
                        nc.vector.tensor_scalar_add(
                            tb[:], ptail[:, 2 * e:2 * e + 1], b1sb[:, e:e + 1]
                        )
                        nc.vector.scalar_tensor_tensor(
                            out=o1T[e][:, 0:1], in0=tb[:],
                            scalar=auxsb[:, nE:nE + 1], in1=auxsb[:, e:e + 1],
                            op0=MUL, op1=ADD,
                        )

                # ---- conv2 + residual + RMSNorm per chunk
                for cl in range(NCL):
                    J0g = h * H + cl * NCH   # global output col
                    J0 = cl * NCH            # local o1T col
                    pssq = ps_ssq.tile([128, NCH], f32, tag="ps_ssq", name="ps_ssq")
                    ys = []
                    for dout in range(nD):
                        wb2 = w2p.tile([128, 2 * nE, 128], f32r, tag="w2b", name="w2b")
                        nc.scalar.dma_start(
                            out=wb2[:],
                            in_=w2pk.ap()[dout].rearrange("t e p m -> p (t e) m"),
                        )
                        py = ps_y.tile([128, NCH], f32, tag="ps_y", name="ps_y")
                        first = True
                        for e in range(nE):
                            for tap in (0, 1):
                                nc.tensor.matmul(
                                    py[:], wb2[:, tap * nE + e, :],
                                    o1T[e][:, J0 + tap:J0 + tap + NCH],
                                    start=first, stop=(e == nE - 1 and tap == 1),
                                )
                                first = False
                        # y = (psum + b2) + x
                        yt = yp.tile([128, NCH], f32, tag=f"y_{dout}", name=f"y_{dout}")
                        k0 = cl * NCH + 2
                        nc.vector.scalar_tensor_tensor(
                            out=yt[:], in0=py[:], scalar=b2sb[:, dout:dout + 1],
                            in1=xh[dout][:, k0:k0 + NCH].bitcast(f32),
                            op0=ADD, op1=ADD,
                        )
                        ys.append(yt)
                        ysq = ysqp.tile([128, NCH], bf16, tag="ysq", name="ysq")
                        nc.vector.tensor_mul(ysq[:], yt[:], yt[:])
                        nc.tensor.matmul(
                            pssq[:], ones[:], ysq[:],
                            start=(dout == 0), stop=(dout == nD - 1),
                        )
                    # inv_rms = 1/sqrt(mean + eps), replicated on all partitions
                    st = invp.tile([128, NCH], f32, tag="st", name="st")
                    nc.scalar.activation(
                        out=st[:], in_=pssq[:],
                        func=mybir.ActivationFunctionType.Sqrt,
                        bias=epssb[:], scale=1.0 / D_,
                    )
                    inv = invp.tile([128, NCH], f32, tag="inv", name="inv")
                    nc.vector.reciprocal(inv[:], st[:])
                    for dout in range(nD):
                        ot = outp.tile([128, NCH], f32, tag="ot", name="ot")
                        nc.vector.scalar_tensor_tensor(
                            out=ot[:], in0=ys[dout][:], scalar=lnwsb[:, dout:dout + 1],
                            in1=inv[:], op0=MUL, op1=MUL,
                        )
                        nc.sync.dma_start(
                            out=outT.ap()[dout * 128:(dout + 1) * 128, J0g:J0g + NCH],
                            in_=ot[:],
                        )

                if h == 0:
                    # chain the half boundary: o1(t0+H-1) -> col 0 for half 1
                    for e in range(nE):
                        nc.vector.tensor_copy(o1T[e][:, 0:1], o1T[e][:, H:H + 1])

            # last conv1 state (token t0+T-1) for the lf2 cache output
            for e in range(nE):
                nc.sync.dma_start(
                    out=o1last.ap().rearrange("(e p) -> p e", p=128)[:, e:e + 1],
                    in_=o1T[e][:, H:H + 1].bitcast(f32),
                )

    nc.compile()
    return nc


# ------------------------------------------------------------------ host glue


def prepare_core_inputs(x3, lf1_cache, lf2_cache, w1, b1, w2, b2, ln_w,
                        ncores, S_, D_, DH_):
    """Build per-core in_maps. x3: [B, S, D] float32."""
    nD = D_ // 128
    nE = DH_ // 128
    B_ = x3.shape[0]
    T_ = (B_ * S_) // ncores
    per_seq = S_ // T_  # cores per sequence

    # lhsT tile-major packs (see build_module comments)
    w1pk = np.ascontiguousarray(
        w1.reshape(nE, 128, nD, 128, 2).transpose(0, 4, 2, 3, 1).astype(np.float32)
    )
    w2pk = np.ascontiguousarray(
        w2.reshape(nD, 128, nE, 128, 2).transpose(0, 4, 2, 3, 1).astype(np.float32)
    )
    b1c = np.ascontiguousarray(b1, np.float32)
    b2c = np.ascontiguousarray(b2, np.float32)
    lnc = np.ascontiguousarray(ln_w, np.float32)

    in_maps = []
    for c in range(ncores):
        b = c // per_seq
        part = c % per_seq
        t0 = part * T_
        x_ext = np.empty((T_ + 2, D_), np.float32)
        aux = np.zeros((128, nE + 1), np.float32)
        if part == 0:
            x_ext[0] = 0.0
            x_ext[1] = lf1_cache[b, :, 0, 0]
            aux[:, :nE] = lf2_cache[b, :, 0, 0].reshape(nE, 128).T
            aux[:, nE] = 0.0
        else:
            x_ext[0] = x3[b, t0 - 2]
            x_ext[1] = x3[b, t0 - 1]
            aux[:, nE] = 1.0
        x_ext[2:] = x3[b, t0:t0 + T_]
        xT = np.ascontiguousarray(x_ext.T)
        in_maps.append({
            "xT": xT, "w1pk": w1pk, "w2pk": w2pk,
            "b1v": b1c, "b2v": b2c, "lnwv": lnc, "aux": aux,
        })
    return in_maps


_CACHE = {}


def _get_module():
    key = (D, DH, T)
    if key not in _CACHE:
        _CACHE[key] = build_module(D, DH, T, 512)
    return _CACHE[key]


def kernel(inputs, lf1_cache, lf2_cache, w1, b1, w2, b2, ln_w):
    from concourse.bass_utils import run_bass_kernel_spmd

    x = np.asarray(inputs, np.float32)
    lf1_cache = np.asarray(lf1_cache, np.float32)
    lf2_cache = np.asarray(lf2_cache, np.float32)
    w1 = np.asarray(w1, np.float32)
    b1 = np.asarray(b1, np.float32)
    w2 = np.asarray(w2, np.float32)
    b2 = np.asarray(b2, np.float32)
    ln_w = np.asarray(ln_w, np.float32)

    x3 = x.reshape(B, S, D)
    in_maps = prepare_core_inputs(x3, lf1_cache, lf2_cache, w1, b1, w2, b2,
                                  ln_w, NCORES, S, D, DH)
    nc = _get_module()
    res = run_bass_kernel_spmd(nc, in_maps, core_ids=list(range(NCORES)))

    per_seq = S // T
    lf_output = np.empty((B, S, D), np.float32)
    lf2 = np.empty((B, DH, 1, 1), np.float32)
    for c in range(NCORES):
        b = c // per_seq
        part = c % per_seq
        t0 = part * T
        lf_output[b, t0:t0 + T] = res.results[c]["outT"].T
        if part == per_seq - 1:
            lf2[b, :, 0, 0] = res.results[c]["o1last"]
    lf1 = np.ascontiguousarray(x3[:, -1][:, :, None, None])
    return lf_output, lf1, lf2


# revision 11
# speedup vs baseline: 1.8682x; 1.0647x over previous
"""LocalizedFiltering (conv1->conv2->residual->RMSNorm) TRN2 Bass kernel.

Full inputs in, full outputs out. Internally: data-parallel over 8 NeuronCores,
2048 tokens per core (each of the 4 sequences of 4096 tokens is split in half;
even cores take sequence starts, odd cores the second halves).

Device layout is channel-major (tokens on the free dim), so matmul contraction
(channels) sits on partitions for both operands. Host transposes per-core input
slabs and output slabs. Matmuls run as float32r (TF32-class rounding, full PE
rate at N=512). The causal kernel-size-2 convs need one previous token (x) and
one previous conv1 output (o1) per shard: previous-x rows come in via the input
slab; previous-o1 is computed on device from the two extra x rows (N=2 matmuls
folded into the conv1 weight loop) and blended against the lf2 cache with a
per-core 0/1 scalar so sequence-start cores use the cache instead. RMSNorm's
cross-partition sum uses a ones-matmul (bf16) accumulating into PSUM, which
also replicates the per-token sum across all partitions for the final scale.
"""

import numpy as np
from contextlib import ExitStack

NCORES = 8
B, S, D = 4, 4096, 2048
DH = D // 2
T = (B * S) // NCORES  # tokens per core
EPS = 1e-6


# ---------------------------------------------------------------- device code


def build_module(D_, DH_, T_, NCH, EPS_CONST=EPS):
    """Build + compile the per-core Bass module. All dims in channel units;
    NCH = token chunk width for matmuls (free dim)."""
    import concourse.tile as tile
    from concourse import bacc, mybir

    f32 = mybir.dt.float32
    f32r = mybir.dt.float32r
    bf16 = mybir.dt.bfloat16
    ADD = mybir.AluOpType.add
    MUL = mybir.AluOpType.mult

    nD = D_ // 128   # input-channel tiles (16)
    nE = DH_ // 128  # hidden-channel tiles (8)
    H = T_ // 2      # half size (1024)
    NCL = H // NCH   # chunks per half (2)

    nc = bacc.Bacc("TRN2", target_bir_lowering=False, debug=False)

    xT = nc.dram_tensor("xT", [D_, T_ + 2], f32r, kind="ExternalInput")
    # weights pre-packed on host, lhsT tile-major:
    # w1pk[e, tap, d, p, m] = w1[e*128+m, d*128+p, tap]
    w1pk = nc.dram_tensor("w1pk", [nE, 2, nD, 128, 128], f32r, kind="ExternalInput")
    # w2pk[do, tap, e, p, m] = w2[do*128+m, e*128+p, tap]
    w2pk = nc.dram_tensor("w2pk", [nD, 2, nE, 128, 128], f32r, kind="ExternalInput")
    b1v = nc.dram_tensor("b1v", [DH_], f32, kind="ExternalInput")
    b2v = nc.dram_tensor("b2v", [D_], f32, kind="ExternalInput")
    lnwv = nc.dram_tensor("lnwv", [D_], f32, kind="ExternalInput")
    # aux[:, e] = o1 state for token t0-1, hidden tile e (start cores: lf2
    # cache; mid cores: host-computed single-token conv1)
    aux = nc.dram_tensor("aux", [128, nE], f32, kind="ExternalInput")

    outT = nc.dram_tensor("outT", [D_, T_], f32, kind="ExternalOutput")
    o1last = nc.dram_tensor("o1last", [DH_], f32, kind="ExternalOutput")

    with tile.TileContext(nc) as tc:
        with ExitStack() as ctx:
            const = ctx.enter_context(tc.tile_pool(name="const", bufs=1))
            o1p = ctx.enter_context(tc.tile_pool(name="o1p", bufs=1))
            xhp = ctx.enter_context(tc.tile_pool(name="xhp", bufs=1))
            w1p = ctx.enter_context(tc.tile_pool(name="w1p", bufs=2))
            w2p = ctx.enter_context(tc.tile_pool(name="w2p", bufs=4))
            yp = ctx.enter_context(tc.tile_pool(name="yp", bufs=1))
            stp = ctx.enter_context(tc.tile_pool(name="stp", bufs=1))
            invp = ctx.enter_context(tc.tile_pool(name="invp", bufs=2))
            ysqp = ctx.enter_context(tc.tile_pool(name="ysqp", bufs=1))
            sqp = ctx.enter_context(tc.tile_pool(name="sqp", bufs=1))
            outp = ctx.enter_context(tc.tile_pool(name="outp", bufs=2))
            ps_o1 = ctx.enter_context(tc.tile_pool(name="ps_o1", bufs=2, space="PSUM"))
            ps_y = ctx.enter_context(tc.tile_pool(name="ps_y", bufs=4, space="PSUM"))
            ps_ssq = ctx.enter_context(tc.tile_pool(name="ps_ssq", bufs=2, space="PSUM"))

            # constants
            b1sb = const.tile([128, nE], f32, tag="b1sb")
            b2sb = const.tile([128, nD], f32, tag="b2sb")
            lnwsb = const.tile([128, nD], f32, tag="lnwsb")
            auxsb = const.tile([128, nE], f32, tag="auxsb")
            ones = const.tile([128, 128], f32, tag="ones")
            epssb = const.tile([128, 1], f32, tag="epssb")
            nc.vector.memset(epssb[:], EPS_CONST)
            nc.sync.dma_start(out=b1sb[:], in_=b1v.ap().rearrange("(e p) -> p e", p=128))
            nc.sync.dma_start(out=b2sb[:], in_=b2v.ap().rearrange("(e p) -> p e", p=128))
            nc.sync.dma_start(out=lnwsb[:], in_=lnwv.ap().rearrange("(e p) -> p e", p=128))
            nc.sync.dma_start(out=auxsb[:], in_=aux.ap())
            nc.vector.memset(ones[:], 1.0)

            # conv1 output for the current half, channel-major [DH_, H+1];
            # col 0 = previous token's o1 (blend for h0, chained from col H after)
            o1T = [o1p.tile([128, H + 1], f32r, tag=f"o1_{e}", name=f"o1_{e}")
                   for e in range(nE)]

            # previous-token o1 state into col 0 of each o1T tile
            for e in range(nE):
                nc.vector.tensor_copy(o1T[e][:, 0:1], auxsb[:, e:e + 1])

            # ~3.4us of dummy matmuls so PE_HAM un-throttles before the DMAs
            # land and real matmuls begin (f32: 4 cycles/row)
            warm = ps_ssq.tile([128, NCH], f32, tag="ps_ssq", name="warm")
            wN = min(128, NCH)
            for i in range(8):
                nc.tensor.matmul(warm[:, 0:wN], ones[:], ones[:, 0:wN],
                                 start=(i == 0), stop=(i == 7))

            for h in range(2):
                # ---- load x half: cols [h*H, h*H + H + 2) of xT
                xh = []
                for d in range(nD):
                    t = xhp.tile([128, H + 2], f32r, tag=f"xh_{d}", name=f"xh_{d}")
                    nc.sync.dma_start(
                        out=t[:], in_=xT.ap()[d * 128:(d + 1) * 128, h * H: h * H + H + 2]
                    )
                    xh.append(t)

                # ---- conv1: o1T local cols [1 + cl*NCH, +NCH) per chunk
                for e in range(nE):
                    wb = w1p.tile([128, 2 * nD, 128], f32r, tag="w1b", name="w1b")
                    nc.gpsimd.dma_start(
                        out=wb[:], in_=w1pk.ap()[e].rearrange("t d p m -> p (t d) m")
                    )
                    pss = [ps_o1.tile([128, NCH], f32, tag="ps_o1", name="ps_o1")
                           for _ in range(NCL)]
                    first = True
                    for d in range(nD):
                        for tap in (0, 1):
                            wt = wb[:, tap * nD + d, :]
                            for cl in range(NCL):
                                k0 = cl * NCH + 1 + tap
                                nc.tensor.matmul(
                                    pss[cl][:], wt, xh[d][:, k0:k0 + NCH],
                                    start=first, stop=(d == nD - 1 and tap == 1),
                                )
                            first = False
                    for cl in range(NCL):
                        nc.vector.tensor_scalar_add(
                            o1T[e][:, 1 + cl * NCH:1 + (cl + 1) * NCH], pss[cl][:],
                            b1sb[:, e:e + 1],
                        )

                # ---- conv2 + residual + RMSNorm per chunk
                for cl in range(NCL):
                    J0g = h * H + cl * NCH   # global output col
                    J0 = cl * NCH            # local o1T col
                    pssq = ps_ssq.tile([128, NCH], f32, tag="ps_ssq", name="ps_ssq")
                    ssqacc = sqp.tile([128, NCH], f32, tag="ssqacc", name="ssqacc")
                    ys = []
                    for dout in range(nD):
                        wb2 = w2p.tile([128, 2 * nE, 128], f32r, tag="w2b", name="w2b")
                        nc.scalar.dma_start(
                            out=wb2[:],
                            in_=w2pk.ap()[dout].rearrange("t e p m -> p (t e) m"),
                        )
                        py = ps_y.tile([128, NCH], f32, tag="ps_y", name="ps_y")
                        first = True
                        for e in range(nE):
                            for tap in (0, 1):
                                nc.tensor.matmul(
                                    py[:], wb2[:, tap * nE + e, :],
                                    o1T[e][:, J0 + tap:J0 + tap + NCH],
                                    start=first, stop=(e == nE - 1 and tap == 1),
                                )
                                first = False
                        # y = (psum + b2) + x
                        yt = yp.tile([128, NCH], f32, tag=f"y_{dout}", name=f"y_{dout}")
                        k0 = cl * NCH + 2
                        nc.vector.scalar_tensor_tensor(
                            out=yt[:], in0=py[:], scalar=b2sb[:, dout:dout + 1],
                            in1=xh[dout][:, k0:k0 + NCH].bitcast(f32),
                            op0=ADD, op1=ADD,
                        )
                        ys.append(yt)
                        if dout == 0:
                            nc.vector.tensor_mul(ssqacc[:], yt[:], yt[:])
                        else:
                            ysq = ysqp.tile([128, NCH], f32, tag="ysq", name="ysq")
                            nc.vector.tensor_mul(ysq[:], yt[:], yt[:])
                            nc.vector.tensor_add(ssqacc[:], ssqacc[:], ysq[:])
                    # cross-partition sum, replicated to all partitions (f32 MM)
                    nc.tensor.matmul(pssq[:], ones[:], ssqacc[:],
                                     start=True, stop=True)
                    # inv_rms = 1/sqrt(mean + eps), replicated on all partitions
                    st = stp.tile([128, NCH], f32, tag="st", name="st")
                    nc.scalar.activation(
                        out=st[:], in_=pssq[:],
                        func=mybir.ActivationFunctionType.Sqrt,
                        bias=epssb[:], scale=1.0 / D_,
                    )
                    inv = invp.tile([128, NCH], f32, tag="inv", name="inv")
                    nc.vector.reciprocal(inv[:], st[:])
                    for dout in range(nD):
                        ot = outp.tile([128, NCH], f32, tag="ot", name="ot")
                        nc.vector.scalar_tensor_tensor(
                            out=ot[:], in0=ys[dout][:], scalar=lnwsb[:, dout:dout + 1],
                            in1=inv[:], op0=MUL, op1=MUL,
                        )
                        nc.sync.dma_start(
                            out=outT.ap()[dout * 128:(dout + 1) * 128, J0g:J0g + NCH],
                            in_=ot[:],
                        )

                if h == 0:
                    # chain the half boundary: o1(t0+H-1) -> col 0 for half 1
                    for e in range(nE):
                        nc.vector.tensor_copy(o1T[e][:, 0:1], o1T[e][:, H:H + 1])

            # last conv1 state (token t0+T-1) for the lf2 cache output
            for e in range(nE):
                nc.sync.dma_start(
                    out=o1last.ap().rearrange("(e p) -> p e", p=128)[:, e:e + 1],
                    in_=o1T[e][:, H:H + 1].bitcast(f32),
                )

    nc.compile()
    return nc


# ------------------------------------------------------------------ host glue


def prepare_core_inputs(x3, lf1_cache, lf2_cache, w1, b1, w2, b2, ln_w,
                        ncores, S_, D_, DH_):
    """Build per-core in_maps. x3: [B, S, D] float32."""
    nD = D_ // 128
    nE = DH_ // 128
    B_ = x3.shape[0]
    T_ = (B_ * S_) // ncores
    per_seq = S_ // T_  # cores per sequence

    # lhsT tile-major packs (see build_module comments)
    w1pk = np.ascontiguousarray(
        w1.reshape(nE, 128, nD, 128, 2).transpose(0, 4, 2, 3, 1).astype(np.float32)
    )
    w2pk = np.ascontiguousarray(
        w2.reshape(nD, 128, nE, 128, 2).transpose(0, 4, 2, 3, 1).astype(np.float32)
    )
    b1c = np.ascontiguousarray(b1, np.float32)
    b2c = np.ascontiguousarray(b2, np.float32)
    lnc = np.ascontiguousarray(ln_w, np.float32)

    in_maps = []
    for c in range(ncores):
        b = c // per_seq
        part = c % per_seq
        t0 = part * T_
        x_ext = np.empty((T_ + 2, D_), np.float32)
        if part == 0:
            x_ext[0] = 0.0
            x_ext[1] = lf1_cache[b, :, 0, 0]
            o1_prev = lf2_cache[b, :, 0, 0]
        else:
            x_ext[0] = x3[b, t0 - 2]
            x_ext[1] = x3[b, t0 - 1]
            # single-token conv1 for the shard-boundary o1 state
            o1_prev = (w1[:, :, 0].astype(np.float32) @ x_ext[0]
                       + w1[:, :, 1].astype(np.float32) @ x_ext[1]
                       + b1.astype(np.float32))
        aux = np.ascontiguousarray(o1_prev.reshape(nE, 128).T.astype(np.float32))
        x_ext[2:] = x3[b, t0:t0 + T_]
        xT = np.ascontiguousarray(x_ext.T)
        in_maps.append({
            "xT": xT, "w1pk": w1pk, "w2pk": w2pk,
            "b1v": b1c, "b2v": b2c, "lnwv": lnc, "aux": aux,
        })
    return in_maps


_CACHE = {}


def _get_module():
    key = (D, DH, T)
    if key not in _CACHE:
        _CACHE[key] = build_module(D, DH, T, 512)
    return _CACHE[key]


def kernel(inputs, lf1_cache, lf2_cache, w1, b1, w2, b2, ln_w):
    from concourse.bass_utils import run_bass_kernel_spmd

    x = np.asarray(inputs, np.float32)
    lf1_cache = np.asarray(lf1_cache, np.float32)
    lf2_cache = np.asarray(lf2_cache, np.float32)
    w1 = np.asarray(w1, np.float32)
    b1 = np.asarray(b1, np.float32)
    w2 = np.asarray(w2, np.float32)
    b2 = np.asarray(b2, np.float32)
    ln_w = np.asarray(ln_w, np.float32)

    x3 = x.reshape(B, S, D)
    in_maps = prepare_core_inputs(x3, lf1_cache, lf2_cache, w1, b1, w2, b2,
                                  ln_w, NCORES, S, D, DH)
    nc = _get_module()
    res = run_bass_kernel_spmd(nc, in_maps, core_ids=list(range(NCORES)))

    per_seq = S // T
    lf_output = np.empty((B, S, D), np.float32)
    lf2 = np.empty((B, DH, 1, 1), np.float32)
    for c in range(NCORES):
        b = c // per_seq
        part = c % per_seq
        t0 = part * T
        lf_output[b, t0:t0 + T] = res.results[c]["outT"].T
        if part == per_seq - 1:
            lf2[b, :, 0, 0] = res.results[c]["o1last"]
    lf1 = np.ascontiguousarray(x3[:, -1][:, :, None, None])
    return lf_output, lf1, lf2


# revision 12
# speedup vs baseline: 2.0317x; 1.0876x over previous
"""LocalizedFiltering (conv1->conv2->residual->RMSNorm) TRN2 Bass kernel.

Full inputs in, full outputs out. Internally: data-parallel over 8 NeuronCores,
2048 tokens per core (each of the 4 sequences of 4096 tokens is split in half;
even cores take sequence starts, odd cores the second halves).

Device layout is channel-major (tokens on the free dim), so matmul contraction
(channels) sits on partitions for both operands. Host transposes per-core input
slabs and output slabs. Matmuls run as float32r (TF32-class rounding, full PE
rate at N=512). The causal kernel-size-2 convs need one previous token (x) and
one previous conv1 output (o1) per shard: previous-x rows come in via the input
slab; previous-o1 is computed on device from the two extra x rows (N=2 matmuls
folded into the conv1 weight loop) and blended against the lf2 cache with a
per-core 0/1 scalar so sequence-start cores use the cache instead. RMSNorm's
cross-partition sum uses a ones-matmul (bf16) accumulating into PSUM, which
also replicates the per-token sum across all partitions for the final scale.
"""

import numpy as np
from contextlib import ExitStack

NCORES = 8
B, S, D = 4, 4096, 2048
DH = D // 2
T = (B * S) // NCORES  # tokens per core
EPS = 1e-6


# ---------------------------------------------------------------- device code


def build_module(D_, DH_, T_, NCH, EPS_CONST=EPS):
    """Build + compile the per-core Bass module. All dims in channel units;
    NCH = token chunk width for matmuls (free dim)."""
    import concourse.tile as tile
    from concourse import bacc, mybir

    f32 = mybir.dt.float32
    f32r = mybir.dt.float32r
    bf16 = mybir.dt.bfloat16
    ADD = mybir.AluOpType.add
    MUL = mybir.AluOpType.mult

    nD = D_ // 128   # input-channel tiles (16)
    nE = DH_ // 128  # hidden-channel tiles (8)
    H = T_ // 2      # half size (1024)
    NCL = H // NCH   # chunks per half (2)

    nc = bacc.Bacc("TRN2", target_bir_lowering=False, debug=False)

    xT = nc.dram_tensor("xT", [D_, T_ + 2], f32r, kind="ExternalInput")
    # weights pre-packed on host, lhsT tile-major:
    # w1pk[e, tap, d, p, m] = w1[e*128+m, d*128+p, tap]
    w1pk = nc.dram_tensor("w1pk", [nE, 2, nD, 128, 128], f32r, kind="ExternalInput")
    # w2pk[do, tap, e, p, m] = w2[do*128+m, e*128+p, tap]
    w2pk = nc.dram_tensor("w2pk", [nD, 2, nE, 128, 128], f32r, kind="ExternalInput")
    b1v = nc.dram_tensor("b1v", [DH_], f32, kind="ExternalInput")
    b2v = nc.dram_tensor("b2v", [D_], f32, kind="ExternalInput")
    lnwv = nc.dram_tensor("lnwv", [D_], f32, kind="ExternalInput")
    # aux[:, e] = o1 state for token t0-1, hidden tile e (start cores: lf2
    # cache; mid cores: host-computed single-token conv1)
    aux = nc.dram_tensor("aux", [128, nE], f32, kind="ExternalInput")

    outT = nc.dram_tensor("outT", [D_, T_], f32, kind="ExternalOutput")
    o1last = nc.dram_tensor("o1last", [DH_], f32, kind="ExternalOutput")

    with tile.TileContext(nc) as tc:
        with ExitStack() as ctx:
            const = ctx.enter_context(tc.tile_pool(name="const", bufs=1))
            o1p = ctx.enter_context(tc.tile_pool(name="o1p", bufs=1))
            xhp = ctx.enter_context(tc.tile_pool(name="xhp", bufs=1))
            w1p = ctx.enter_context(tc.tile_pool(name="w1p", bufs=4))
            w2p = ctx.enter_context(tc.tile_pool(name="w2p", bufs=8))
            yp = ctx.enter_context(tc.tile_pool(name="yp", bufs=1))
            stp = ctx.enter_context(tc.tile_pool(name="stp", bufs=1))
            invp = ctx.enter_context(tc.tile_pool(name="invp", bufs=2))
            ysqp = ctx.enter_context(tc.tile_pool(name="ysqp", bufs=1))
            sqp = ctx.enter_context(tc.tile_pool(name="sqp", bufs=1))
            outp = ctx.enter_context(tc.tile_pool(name="outp", bufs=2))
            ps_o1 = ctx.enter_context(tc.tile_pool(name="ps_o1", bufs=2, space="PSUM"))
            ps_y = ctx.enter_context(tc.tile_pool(name="ps_y", bufs=4, space="PSUM"))
            ps_ssq = ctx.enter_context(tc.tile_pool(name="ps_ssq", bufs=2, space="PSUM"))

            # constants
            b1sb = const.tile([128, nE], f32, tag="b1sb")
            b2sb = const.tile([128, nD], f32, tag="b2sb")
            lnwsb = const.tile([128, nD], f32, tag="lnwsb")
            auxsb = const.tile([128, nE], f32, tag="auxsb")
            ones = const.tile([128, 128], f32, tag="ones")
            epssb = const.tile([128, 1], f32, tag="epssb")
            nc.vector.memset(epssb[:], EPS_CONST)
            nc.sync.dma_start(out=b1sb[:], in_=b1v.ap().rearrange("(e p) -> p e", p=128))
            nc.sync.dma_start(out=b2sb[:], in_=b2v.ap().rearrange("(e p) -> p e", p=128))
            nc.sync.dma_start(out=lnwsb[:], in_=lnwv.ap().rearrange("(e p) -> p e", p=128))
            nc.sync.dma_start(out=auxsb[:], in_=aux.ap())
            nc.vector.memset(ones[:], 1.0)

            # conv1 output for the current half, channel-major [DH_, H+1];
            # col 0 = previous token's o1 (blend for h0, chained from col H after)
            o1T = [o1p.tile([128, H + 1], f32r, tag=f"o1_{e}", name=f"o1_{e}")
                   for e in range(nE)]

            # previous-token o1 state into col 0 of each o1T tile
            for e in range(nE):
                nc.vector.tensor_copy(o1T[e][:, 0:1], auxsb[:, e:e + 1])

            # ~3.4us of dummy matmuls so PE_HAM un-throttles before the DMAs
            # land and real matmuls begin (f32: 4 cycles/row)
            warm = ps_ssq.tile([128, NCH], f32, tag="ps_ssq", name="warm")
            wN = min(128, NCH)
            for i in range(8):
                nc.tensor.matmul(warm[:, 0:wN], ones[:], ones[:, 0:wN],
                                 start=(i == 0), stop=(i == 7))

            for h in range(2):
                # ---- load x half: cols [h*H, h*H + H + 2) of xT
                xh = []
                for d in range(nD):
                    t = xhp.tile([128, H + 2], f32r, tag=f"xh_{d}", name=f"xh_{d}")
                    nc.sync.dma_start(
                        out=t[:], in_=xT.ap()[d * 128:(d + 1) * 128, h * H: h * H + H + 2]
                    )
                    xh.append(t)

                # ---- conv1: o1T local cols [1 + cl*NCH, +NCH) per chunk
                for e in range(nE):
                    wb = []
                    for tap in (0, 1):
                        w = w1p.tile([128, nD, 128], f32r, tag="w1b", name="w1b")
                        nc.gpsimd.dma_start(
                            out=w[:], in_=w1pk.ap()[e, tap].rearrange("d p m -> p d m")
                        )
                        wb.append(w)
                    pss = [ps_o1.tile([128, NCH], f32, tag="ps_o1", name="ps_o1")
                           for _ in range(NCL)]
                    first = True
                    for d in range(nD):
                        for tap in (0, 1):
                            wt = wb[tap][:, d, :]
                            for cl in range(NCL):
                                k0 = cl * NCH + 1 + tap
                                nc.tensor.matmul(
                                    pss[cl][:], wt, xh[d][:, k0:k0 + NCH],
                                    start=first, stop=(d == nD - 1 and tap == 1),
                                )
                            first = False
                    for cl in range(NCL):
                        nc.vector.tensor_scalar_add(
                            o1T[e][:, 1 + cl * NCH:1 + (cl + 1) * NCH], pss[cl][:],
                            b1sb[:, e:e + 1],
                        )

                # ---- conv2 + residual + RMSNorm per chunk
                for cl in range(NCL):
                    J0g = h * H + cl * NCH   # global output col
                    J0 = cl * NCH            # local o1T col
                    pssq = ps_ssq.tile([128, NCH], f32, tag="ps_ssq", name="ps_ssq")
                    ssqacc = sqp.tile([128, NCH], f32, tag="ssqacc", name="ssqacc")
                    ys = []
                    for dout in range(nD):
                        wb2 = []
                        for tap in (0, 1):
                            w = w2p.tile([128, nE, 128], f32r, tag="w2b", name="w2b")
                            nc.scalar.dma_start(
                                out=w[:],
                                in_=w2pk.ap()[dout, tap].rearrange("e p m -> p e m"),
                            )
                            wb2.append(w)
                        py = ps_y.tile([128, NCH], f32, tag="ps_y", name="ps_y")
                        first = True
                        for e in range(nE):
                            for tap in (0, 1):
                                nc.tensor.matmul(
                                    py[:], wb2[tap][:, e, :],
                                    o1T[e][:, J0 + tap:J0 + tap + NCH],
                                    start=first, stop=(e == nE - 1 and tap == 1),
                                )
                                first = False
                        # y = (psum + b2) + x
                        yt = yp.tile([128, NCH], f32, tag=f"y_{dout}", name=f"y_{dout}")
                        k0 = cl * NCH + 2
                        nc.vector.scalar_tensor_tensor(
                            out=yt[:], in0=py[:], scalar=b2sb[:, dout:dout + 1],
                            in1=xh[dout][:, k0:k0 + NCH].bitcast(f32),
                            op0=ADD, op1=ADD,
                        )
                        ys.append(yt)
                        if dout == 0:
                            nc.vector.tensor_mul(ssqacc[:], yt[:], yt[:])
                        else:
                            ysq = ysqp.tile([128, NCH], f32, tag="ysq", name="ysq")
                            nc.vector.tensor_mul(ysq[:], yt[:], yt[:])
                            nc.vector.tensor_add(ssqacc[:], ssqacc[:], ysq[:])
                    # cross-partition sum, replicated to all partitions (f32 MM)
                    nc.tensor.matmul(pssq[:], ones[:], ssqacc[:],
                                     start=True, stop=True)
                    # inv_rms = 1/sqrt(mean + eps), replicated on all partitions
                    st = stp.tile([128, NCH], f32, tag="st", name="st")
                    nc.scalar.activation(
                        out=st[:], in_=pssq[:],
                        func=mybir.ActivationFunctionType.Sqrt,
                        bias=epssb[:], scale=1.0 / D_,
                    )
                    inv = invp.tile([128, NCH], f32, tag="inv", name="inv")
                    nc.vector.reciprocal(inv[:], st[:])
                    for dout in range(nD):
                        ot = outp.tile([128, NCH], f32, tag="ot", name="ot")
                        nc.vector.scalar_tensor_tensor(
                            out=ot[:], in0=ys[dout][:], scalar=lnwsb[:, dout:dout + 1],
                            in1=inv[:], op0=MUL, op1=MUL,
                        )
                        nc.sync.dma_start(
                            out=outT.ap()[dout * 128:(dout + 1) * 128, J0g:J0g + NCH],
                            in_=ot[:],
                        )

                if h == 0:
                    # chain the half boundary: o1(t0+H-1) -> col 0 for half 1
                    for e in range(nE):
                        nc.vector.tensor_copy(o1T[e][:, 0:1], o1T[e][:, H:H + 1])

            # last conv1 state (token t0+T-1) for the lf2 cache output
            for e in range(nE):
                nc.sync.dma_start(
                    out=o1last.ap().rearrange("(e p) -> p e", p=128)[:, e:e + 1],
                    in_=o1T[e][:, H:H + 1].bitcast(f32),
                )

    nc.compile()
    return nc


# ------------------------------------------------------------------ host glue


def prepare_core_inputs(x3, lf1_cache, lf2_cache, w1, b1, w2, b2, ln_w,
                        ncores, S_, D_, DH_):
    """Build per-core in_maps. x3: [B, S, D] float32."""
    nD = D_ // 128
    nE = DH_ // 128
    B_ = x3.shape[0]
    T_ = (B_ * S_) // ncores
    per_seq = S_ // T_  # cores per sequence

    # lhsT tile-major packs (see build_module comments)
    w1pk = np.ascontiguousarray(
        w1.reshape(nE, 128, nD, 128, 2).transpose(0, 4, 2, 3, 1).astype(np.float32)
    )
    w2pk = np.ascontiguousarray(
        w2.reshape(nD, 128, nE, 128, 2).transpose(0, 4, 2, 3, 1).astype(np.float32)
    )
    b1c = np.ascontiguousarray(b1, np.float32)
    b2c = np.ascontiguousarray(b2, np.float32)
    lnc = np.ascontiguousarray(ln_w, np.float32)

    in_maps = []
    for c in range(ncores):
        b = c // per_seq
        part = c % per_seq
        t0 = part * T_
        x_ext = np.empty((T_ + 2, D_), np.float32)
        if part == 0:
            x_ext[0] = 0.0
            x_ext[1] = lf1_cache[b, :, 0, 0]
            o1_prev = lf2_cache[b, :, 0, 0]
        else:
            x_ext[0] = x3[b, t0 - 2]
            x_ext[1] = x3[b, t0 - 1]
            # single-token conv1 for the shard-boundary o1 state
            o1_prev = (w1[:, :, 0].astype(np.float32) @ x_ext[0]
                       + w1[:, :, 1].astype(np.float32) @ x_ext[1]
                       + b1.astype(np.float32))
        aux = np.ascontiguousarray(o1_prev.reshape(nE, 128).T.astype(np.float32))
        x_ext[2:] = x3[b, t0:t0 + T_]
        xT = np.ascontiguousarray(x_ext.T)
        in_maps.append({
            "xT": xT, "w1pk": w1pk, "w2pk": w2pk,
            "b1v": b1c, "b2v": b2c, "lnwv": lnc, "aux": aux,
        })
    return in_maps


_CACHE = {}


def _get_module():
    key = (D, DH, T)
    if key not in _CACHE:
        _CACHE[key] = build_module(D, DH, T, 512)
    return _CACHE[key]


def kernel(inputs, lf1_cache, lf2_cache, w1, b1, w2, b2, ln_w):
    from concourse.bass_utils import run_bass_kernel_spmd

    x = np.asarray(inputs, np.float32)
    lf1_cache = np.asarray(lf1_cache, np.float32)
    lf2_cache = np.asarray(lf2_cache, np.float32)
    w1 = np.asarray(w1, np.float32)
    b1 = np.asarray(b1, np.float32)
    w2 = np.asarray(w2, np.float32)
    b2 = np.asarray(b2, np.float32)
    ln_w = np.asarray(ln_w, np.float32)

    x3 = x.reshape(B, S, D)
    in_maps = prepare_core_inputs(x3, lf1_cache, lf2_cache, w1, b1, w2, b2,
                                  ln_w, NCORES, S, D, DH)
    nc = _get_module()
    res = run_bass_kernel_spmd(nc, in_maps, core_ids=list(range(NCORES)))

    per_seq = S // T
    lf_output = np.empty((B, S, D), np.float32)
    lf2 = np.empty((B, DH, 1, 1), np.float32)
    for c in range(NCORES):
        b = c // per_seq
        part = c % per_seq
        t0 = part * T
        lf_output[b, t0:t0 + T] = res.results[c]["outT"].T
        if part == per_seq - 1:
            lf2[b, :, 0, 0] = res.results[c]["o1last"]
    lf1 = np.ascontiguousarray(x3[:, -1][:, :, None, None])
    return lf_output, lf1, lf2
